# revision 10
# baseline (speedup 1.0000x reference)
"""SPDnet autoencoder (nn_Autoencoder_layers_byhalf_SPDnet) on 8 trn2 NeuronCores.

Mathematical collapse (same as the A x A^T + C baseline, see below), plus a
rank-16 factorization that the device exploits:

  out[b] = A x[b] A^T + C,  A = R L  with  L (16x128) = W2 W1 W0 and
  R (128x16) = D2 D1 D0 both semi-orthogonal;  C is a host constant.

Because A has rank 16, the full product never needs to be formed on device:

  out = sym(A2 (x L~^T) R^T) + C        (up to quantization)

where L~ = dequant(fp8(sl*L))/sl and A2 = fp16(2A - R L~).  The choice
A2 = 2A - R L~ makes the first-order weight-quantization error cancel under
host symmetrization:  sym((A+d) x (A-d)^T) = A x A^T - d x d^T.

Device (per core, 256 SPD matrices, fp8 input):
  * Host packs x -> e3m4(4*x) in SBUF layout [p, (m c)]  (4.19 MB/core,
    half the fp16 baseline's input bytes).
  * Per group of 8 matrices: 8 tiny matmuls V'_b = x_b @ L8T (stationary
    x_b fp8 with auto FWL, moving L8T [128,16] -> stream 16 cycles), one
    whole-tile evac [128,128] f32->fp16, one batched matmul
    W8 = A2 @ [V'_0..V'_7] (stationary A2^T fp16 shared across the group,
    stream 128), one evac to the persistent output tile.
  * Output is only W (128x16 fp16 per matrix) = 1.05 MB/core, 8x less than
    the full symmetric output.
Host: out = sym(W (R/(2*sx*sl))^T)*2... i.e. WR + WR^T + C with the scale
folded into R.  Expansion is one 262144x16 @ 16x128 sgemm + transpose-add.

Accuracy: rel fro err ~7.3e-3 end-to-end (gate 2e-2), dominated by the x
e3m4 quantization (which is attenuated 8x by the rank-16 projection).
"""

import numpy as np

N_CORES = 8
BATCH = 2048
N = 128
K = 16                               # rank of A / W columns
PER_CORE = BATCH // N_CORES          # 256
GROUP = 16                           # SPD matrices per PSUM tile
N_GROUPS = PER_CORE // GROUP         # 16
EPS = 1e-4
SX = 4.0                             # x fp8 scale
SL = 16.0                            # L fp8 scale

_compiled = {}


def _host_consts(w_enc0, w_enc1, w_enc2, w_dec0, w_dec1, w_dec2):
    """Device consts (l8t fp8, a2t fp16) and host expansion mats (Rh, C)."""
    import ml_dtypes

    f8 = np.float64
    W0 = w_enc0[0, 0].astype(f8)     # (64,128)
    W1 = w_enc1[0, 0].astype(f8)     # (32,64)
    W2 = w_enc2[0, 0].astype(f8)     # (16,32)
    D0 = w_dec0[0, 0].astype(f8)     # (32,16)
    D1 = w_dec1[0, 0].astype(f8)     # (64,32)
    D2 = w_dec2[0, 0].astype(f8)     # (128,64)
    L = W2 @ W1 @ W0                 # (16,128)
    R = D2 @ D1 @ D0                 # (128,16)
    A = R @ L                        # (128,128) rank 16
    P1 = np.eye(32) - D0 @ D0.T
    P2 = np.eye(64) - D1 @ D1.T
    P3 = np.eye(128) - D2 @ D2.T
    C = EPS * (D2 @ (D1 @ P1 @ D1.T + P2) @ D2.T + P3)

    l8t = np.ascontiguousarray(SL * L.T).astype(np.float32).astype(
        ml_dtypes.float8_e3m4)                       # (128,16) fp8 device const
    Ltil = l8t.astype(np.float64).T / SL             # dequantized L~
    A2 = (2.0 * A - R @ Ltil).astype(np.float16)     # fp16, error-cancelling
    a2t = np.ascontiguousarray(A2.T)                 # (128,128) fp16
    # host expansion matrix: out = W @ Rh^T + (W @ Rh^T)^T + C
    # W carries scale SX*SL and we also need the 1/2 from sym():
    Rh = (R / (2.0 * SX * SL)).astype(np.float32)    # (128,16)
    return l8t, a2t, Rh, C.astype(np.float32)


def _build_bass(psum_bufs=3):
    import concourse.mybir as mybir
    from concourse import bacc
    from concourse.tile import TileContext

    W = GROUP * K                    # W8 tile width (128)
    WALL = PER_CORE * N              # full-core x width (32768)
    WOUT = PER_CORE * K              # full-core w width (4096)

    nc = bacc.Bacc(None, target_bir_lowering=False)
    f8e3 = mybir.dt.float8e3
    f16 = mybir.dt.float16
    f32 = mybir.dt.float32
    x = nc.dram_tensor("x", [N, WALL], f8e3, kind="ExternalInput")
    l8t = nc.dram_tensor("l8t", [N, K], f8e3, kind="ExternalInput")
    a2t = nc.dram_tensor("a2t", [N, N], f16, kind="ExternalInput")
    wout = nc.dram_tensor("w", [N, WOUT], f16, kind="ExternalOutput")

    # input slice schedule in matrices: fine first so compute starts early.
    # Issue is THROTTLED (~2 slices outstanding): the DMA rings round-robin
    # chunks of all outstanding transfers, so a deep backlog makes every
    # slice complete near the end and stalls compute on early matrices.
    in_sizes = [4, 8, 16, 32, 36, 32, 32, 32, 32, 32]
    # output slice schedule in groups (1 group = GROUP mats = GROUP*K cols)
    out_sizes_g = [8, 4, 2, 1, 1]
    assert sum(in_sizes) == PER_CORE and sum(out_sizes_g) == N_GROUPS

    with TileContext(nc) as tc:
        with (
            tc.tile_pool(name="consts", bufs=1) as cpool,
            tc.tile_pool(name="ysb", bufs=4) as ypool,
            tc.tile_pool(name="psy", bufs=psum_bufs, space="PSUM") as psy_pool,
            tc.tile_pool(name="pso", bufs=psum_bufs, space="PSUM") as pso_pool,
        ):
            l8t_sb = cpool.tile([N, K], f8e3)
            a2t_sb = cpool.tile([N, N], f16)
            nc.gpsimd.dma_start(out=l8t_sb, in_=l8t[:, :])
            nc.gpsimd.dma_start(out=a2t_sb, in_=a2t[:, :])
            xt = cpool.tile([N, WALL], f8e3)     # whole-core input (32KB/part)
            wt = cpool.tile([N, WOUT], f16)      # whole-core output (8KB/part)

            in_done = 0
            in_idx = 0
            LOOKAHEAD = 64                       # matrices issued ahead
            out_done = 0                         # groups drained
            next_out = out_sizes_g[0]
            oi = 0
            ysb_prev = None                      # software pipeline: mm2 lags
            for k in range(N_GROUPS + 1):
                if k < N_GROUPS:
                    while (in_done < PER_CORE
                           and in_done < (k + 1) * GROUP + LOOKAHEAD):
                        sz = in_sizes[in_idx]
                        a, b = in_done * N, (in_done + sz) * N
                        ieng = nc.sync if in_idx % 2 == 0 else nc.gpsimd
                        ieng.dma_start(out=xt[:, a:b], in_=x[:, a:b])
                        in_done += sz
                        in_idx += 1
                # mm2 of the PREVIOUS group FIRST: it is ready work, so it
                # is never stuck in the PE FIFO behind input-waiting mm1s.
                if ysb_prev is not None:
                    pso = pso_pool.tile([N, W], f32, tag="pso")
                    nc.tensor.matmul(
                        pso, lhsT=a2t_sb, rhs=ysb_prev, start=True, stop=True,
                    )
                    nc.vector.tensor_copy(
                        wt[:, (k - 1) * W:k * W], pso)
                if k < N_GROUPS:
                    psy = psy_pool.tile([N, W], f32, tag="psy")
                    for g in range(GROUP):
                        m = k * GROUP + g
                        nc.tensor.matmul(
                            psy[:, g * K:(g + 1) * K],
                            lhsT=xt[:, m * N:(m + 1) * N],
                            rhs=l8t_sb,
                            start=True, stop=True,
                        )
                    ysb = ypool.tile([N, W], f16, tag="ysb")
                    nc.scalar.copy(ysb, psy)
                    ysb_prev = ysb
                # drain completed output slices (group k-1 written above)
                while k >= out_done + next_out:
                    a, b = out_done * W, (out_done + next_out) * W
                    oeng = nc.sync if oi >= len(out_sizes_g) - 2 else nc.gpsimd
                    oeng.dma_start(out=wout[:, a:b], in_=wt[:, a:b])
                    out_done += next_out
                    oi += 1
                    next_out = out_sizes_g[oi] if oi < len(out_sizes_g) else N_GROUPS
    nc.compile()
    return nc


def _pack_x(x_full):
    """(BATCH,N,N) f32 -> per-core fp8 [N, PER_CORE*N] SBUF layout [p,(m c)]."""
    import ml_dtypes

    x8 = (x_full * np.float32(SX)).astype(ml_dtypes.float8_e3m4)
    x8 = x8.reshape(N_CORES, PER_CORE, N, N).transpose(0, 2, 1, 3)
    return np.ascontiguousarray(x8).reshape(N_CORES, N, PER_CORE * N)


def _get_nc():
    if "nc" not in _compiled:
        _compiled["nc"] = _build_bass()
    return _compiled["nc"]


def kernel(x, w_enc0, w_enc1, w_enc2, w_dec0, w_dec1, w_dec2, trace=False):
    from concourse.bass_utils import run_bass_kernel_spmd

    l8t, a2t, Rh, C = _host_consts(
        w_enc0, w_enc1, w_enc2, w_dec0, w_dec1, w_dec2)
    xs = np.asarray(x, dtype=np.float32).reshape(BATCH, N, N)
    xp = _pack_x(xs)

    nc = _get_nc()
    in_maps = [
        {"x": xp[i], "l8t": l8t, "a2t": a2t}
        for i in range(N_CORES)
    ]
    res = run_bass_kernel_spmd(nc, in_maps, core_ids=list(range(N_CORES)), trace=trace)
    # gather W: per core [N, PER_CORE*K] -> (BATCH, N, K)
    Wg = np.concatenate(
        [
            np.ascontiguousarray(
                r["w"].reshape(N, PER_CORE, K).transpose(1, 0, 2))
            for r in res.results
        ],
        axis=0,
    ).astype(np.float32)
    # host expansion: out = W Rh^T + (W Rh^T)^T + C   (scales folded into Rh)
    WR = (Wg.reshape(-1, K) @ Rh.T).reshape(BATCH, N, N)
    out = WR + WR.transpose(0, 2, 1)
    out += C
    if trace:
        _compiled["last_results"] = res
    return out.reshape(BATCH, 1, N, N)


# revision 14
# speedup vs baseline: 1.1623x; 1.1623x over previous
"""SPDnet autoencoder (nn_Autoencoder_layers_byhalf_SPDnet) on 8 trn2 NeuronCores.

Mathematical collapse (same as the A x A^T + C baseline, see below), plus a
rank-16 factorization that the device exploits:

  out[b] = A x[b] A^T + C,  A = R L  with  L (16x128) = W2 W1 W0 and
  R (128x16) = D2 D1 D0 both semi-orthogonal;  C is a host constant.

Because A has rank 16, the full product never needs to be formed on device:

  out = sym(A2 (x L~^T) R^T) + C        (up to quantization)

where L~ = dequant(fp8(sl*L))/sl and A2 = fp16(2A - R L~).  The choice
A2 = 2A - R L~ makes the first-order weight-quantization error cancel under
host symmetrization:  sym((A+d) x (A-d)^T) = A x A^T - d x d^T.

Device (per core, 256 SPD matrices, fp8 input):
  * Host packs x -> e3m4(4*x) in SBUF layout [p, (m c)]  (4.19 MB/core,
    half the fp16 baseline's input bytes).
  * Per group of 8 matrices: 8 tiny matmuls V'_b = x_b @ L8T (stationary
    x_b fp8 with auto FWL, moving L8T [128,16] -> stream 16 cycles), one
    whole-tile evac [128,128] f32->fp16, one batched matmul
    W8 = A2 @ [V'_0..V'_7] (stationary A2^T fp16 shared across the group,
    stream 128), one evac to the persistent output tile.
  * Output is only W (128x16 fp16 per matrix) = 1.05 MB/core, 8x less than
    the full symmetric output.
Host: out = sym(W (R/(2*sx*sl))^T)*2... i.e. WR + WR^T + C with the scale
folded into R.  Expansion is one 262144x16 @ 16x128 sgemm + transpose-add.

Accuracy: rel fro err ~7.3e-3 end-to-end (gate 2e-2), dominated by the x
e3m4 quantization (which is attenuated 8x by the rank-16 projection).
"""

import numpy as np

N_CORES = 8
BATCH = 2048
N = 128
K = 16                               # rank of A / W columns
PER_CORE = BATCH // N_CORES          # 256
GROUP = 16                           # SPD matrices per PSUM tile
N_GROUPS = PER_CORE // GROUP         # 16
EPS = 1e-4
SX = 4.0                             # x fp8 scale
SL = 16.0                            # L fp8 scale

_compiled = {}


def _host_consts(w_enc0, w_enc1, w_enc2, w_dec0, w_dec1, w_dec2):
    """Device consts (l8t fp8, a2t fp16) and host expansion mats (Rh, C)."""
    import ml_dtypes

    f8 = np.float64
    W0 = w_enc0[0, 0].astype(f8)     # (64,128)
    W1 = w_enc1[0, 0].astype(f8)     # (32,64)
    W2 = w_enc2[0, 0].astype(f8)     # (16,32)
    D0 = w_dec0[0, 0].astype(f8)     # (32,16)
    D1 = w_dec1[0, 0].astype(f8)     # (64,32)
    D2 = w_dec2[0, 0].astype(f8)     # (128,64)
    L = W2 @ W1 @ W0                 # (16,128)
    R = D2 @ D1 @ D0                 # (128,16)
    A = R @ L                        # (128,128) rank 16
    P1 = np.eye(32) - D0 @ D0.T
    P2 = np.eye(64) - D1 @ D1.T
    P3 = np.eye(128) - D2 @ D2.T
    C = EPS * (D2 @ (D1 @ P1 @ D1.T + P2) @ D2.T + P3)

    l8t = np.ascontiguousarray(SL * L.T).astype(np.float32).astype(
        ml_dtypes.float8_e3m4)                       # (128,16) fp8 device const
    Ltil = l8t.astype(np.float64).T / SL             # dequantized L~
    A2 = (2.0 * A - R @ Ltil).astype(np.float16)     # fp16, error-cancelling
    a2t = np.ascontiguousarray(A2.T)                 # (128,128) fp16
    # host expansion matrix: out = W @ Rh^T + (W @ Rh^T)^T + C
    # W carries scale SX*SL and we also need the 1/2 from sym():
    Rh = (R / (2.0 * SX * SL)).astype(np.float32)    # (128,16)
    return l8t, a2t, Rh, C.astype(np.float32)


def _build_bass(psum_bufs=3):
    import concourse.mybir as mybir
    from concourse import bacc
    from concourse.tile import TileContext

    W = GROUP * K                    # W8 tile width (128)
    WALL = PER_CORE * N              # full-core x width (32768)
    WOUT = PER_CORE * K              # full-core w width (4096)

    nc = bacc.Bacc(None, target_bir_lowering=False)
    f8e3 = mybir.dt.float8e3
    f16 = mybir.dt.float16
    f32 = mybir.dt.float32
    x = nc.dram_tensor("x", [N, WALL], f8e3, kind="ExternalInput")
    l8t = nc.dram_tensor("l8t", [N, K], f8e3, kind="ExternalInput")
    a2t = nc.dram_tensor("a2t", [N, N], f16, kind="ExternalInput")
    wout = nc.dram_tensor("w", [N, WOUT], f16, kind="ExternalOutput")

    # input slice schedule in matrices: fine first so compute starts early.
    # Issue is THROTTLED (~1-2 slices outstanding): the DMA rings round-robin
    # chunks of all outstanding transfers, so a deep backlog makes every
    # slice complete near the end and stalls compute on early matrices.
    in_sizes = [8, 24, 32, 32, 32, 32, 32, 32, 32]
    # output slice schedule in groups (1 group = GROUP mats = GROUP*K cols)
    out_sizes_g = [8, 4, 2, 1, 1]
    assert sum(in_sizes) == PER_CORE and sum(out_sizes_g) == N_GROUPS

    with TileContext(nc) as tc:
        with (
            tc.tile_pool(name="consts", bufs=1) as cpool,
            tc.tile_pool(name="ysb", bufs=4) as ypool,
            tc.tile_pool(name="psy", bufs=psum_bufs, space="PSUM") as psy_pool,
            tc.tile_pool(name="pso", bufs=psum_bufs, space="PSUM") as pso_pool,
        ):
            l8t_sb = cpool.tile([N, K], f8e3)
            a2t_sb = cpool.tile([N, N], f16)
            nc.scalar.dma_start(out=l8t_sb, in_=l8t[:, :])
            nc.scalar.dma_start(out=a2t_sb, in_=a2t[:, :])
            xt = cpool.tile([N, WALL], f8e3)     # whole-core input (32KB/part)
            wt = cpool.tile([N, WOUT], f16)      # whole-core output (8KB/part)

            in_done = 0
            in_idx = 0
            LOOKAHEAD = 48                       # matrices issued ahead
            out_done = 0                         # groups drained
            next_out = out_sizes_g[0]
            oi = 0
            ysb_prev = None                      # software pipeline: mm2 lags
            for k in range(N_GROUPS + 1):
                if k < N_GROUPS:
                    while (in_done < PER_CORE
                           and in_done < (k + 1) * GROUP + LOOKAHEAD):
                        sz = in_sizes[in_idx]
                        a, b = in_done * N, (in_done + sz) * N
                        nc.sync.dma_start(out=xt[:, a:b], in_=x[:, a:b])
                        in_done += sz
                        in_idx += 1
                    psy = psy_pool.tile([N, W], f32, tag="psy")
                    for g in range(GROUP):
                        m = k * GROUP + g
                        nc.tensor.matmul(
                            psy[:, g * K:(g + 1) * K],
                            lhsT=xt[:, m * N:(m + 1) * N],
                            rhs=l8t_sb,
                            start=True, stop=True,
                        )
                # mm2 of the PREVIOUS group: its ysb is long ready, so the
                # PE never stalls on the PSUM evacuation of group k.
                if ysb_prev is not None:
                    pso = pso_pool.tile([N, W], f32, tag="pso")
                    nc.tensor.matmul(
                        pso, lhsT=a2t_sb, rhs=ysb_prev, start=True, stop=True,
                    )
                    nc.vector.tensor_copy(
                        wt[:, (k - 1) * W:k * W], pso)
                if k < N_GROUPS:
                    ysb = ypool.tile([N, W], f16, tag="ysb")
                    nc.scalar.copy(ysb, psy)
                    ysb_prev = ysb
                # drain completed output slices (group k-1 written above)
                while k >= out_done + next_out:
                    a, b = out_done * W, (out_done + next_out) * W
                    oeng = nc.sync if oi >= len(out_sizes_g) - 2 else nc.gpsimd
                    oeng.dma_start(out=wout[:, a:b], in_=wt[:, a:b])
                    out_done += next_out
                    oi += 1
                    next_out = out_sizes_g[oi] if oi < len(out_sizes_g) else N_GROUPS
    nc.compile()
    return nc


def _pack_x(x_full):
    """(BATCH,N,N) f32 -> per-core fp8 [N, PER_CORE*N] SBUF layout [p,(m c)]."""
    import ml_dtypes

    x8 = (x_full * np.float32(SX)).astype(ml_dtypes.float8_e3m4)
    x8 = x8.reshape(N_CORES, PER_CORE, N, N).transpose(0, 2, 1, 3)
    return np.ascontiguousarray(x8).reshape(N_CORES, N, PER_CORE * N)


def _get_nc():
    if "nc" not in _compiled:
        _compiled["nc"] = _build_bass()
    return _compiled["nc"]


def kernel(x, w_enc0, w_enc1, w_enc2, w_dec0, w_dec1, w_dec2, trace=False):
    from concourse.bass_utils import run_bass_kernel_spmd

    l8t, a2t, Rh, C = _host_consts(
        w_enc0, w_enc1, w_enc2, w_dec0, w_dec1, w_dec2)
    xs = np.asarray(x, dtype=np.float32).reshape(BATCH, N, N)
    xp = _pack_x(xs)

    nc = _get_nc()
    in_maps = [
        {"x": xp[i], "l8t": l8t, "a2t": a2t}
        for i in range(N_CORES)
    ]
    res = run_bass_kernel_spmd(nc, in_maps, core_ids=list(range(N_CORES)), trace=trace)
    # gather W: per core [N, PER_CORE*K] -> (BATCH, N, K)
    Wg = np.concatenate(
        [
            np.ascontiguousarray(
                r["w"].reshape(N, PER_CORE, K).transpose(1, 0, 2))
            for r in res.results
        ],
        axis=0,
    ).astype(np.float32)
    # host expansion: out = W Rh^T + (W Rh^T)^T + C   (scales folded into Rh)
    WR = (Wg.reshape(-1, K) @ Rh.T).reshape(BATCH, N, N)
    out = WR + WR.transpose(0, 2, 1)
    out += C
    if trace:
        _compiled["last_results"] = res
    return out.reshape(BATCH, 1, N, N)


# revision 17
# speedup vs baseline: 1.1883x; 1.0224x over previous
"""SPDnet autoencoder (nn_Autoencoder_layers_byhalf_SPDnet) on 8 trn2 NeuronCores.

Mathematical collapse (same as the A x A^T + C baseline, see below), plus a
rank-16 factorization that the device exploits:

  out[b] = A x[b] A^T + C,  A = R L  with  L (16x128) = W2 W1 W0 and
  R (128x16) = D2 D1 D0 both semi-orthogonal;  C is a host constant.

Because A has rank 16, the full product never needs to be formed on device:

  out = sym(A2 (x L~^T) R^T) + C        (up to quantization)

where L~ = dequant(fp8(sl*L))/sl and A2 = fp16(2A - R L~).  The choice
A2 = 2A - R L~ makes the first-order weight-quantization error cancel under
host symmetrization:  sym((A+d) x (A-d)^T) = A x A^T - d x d^T.

Device (per core, 256 SPD matrices, fp8 input):
  * Host packs x -> e3m4(4*x) in SBUF layout [p, (m c)]  (4.19 MB/core,
    half the fp16 baseline's input bytes).
  * Per group of 8 matrices: 8 tiny matmuls V'_b = x_b @ L8T (stationary
    x_b fp8 with auto FWL, moving L8T [128,16] -> stream 16 cycles), one
    whole-tile evac [128,128] f32->fp16, one batched matmul
    W8 = A2 @ [V'_0..V'_7] (stationary A2^T fp16 shared across the group,
    stream 128), one evac to the persistent output tile.
  * Output is only W (128x16 fp16 per matrix) = 1.05 MB/core, 8x less than
    the full symmetric output.
Host: out = sym(W (R/(2*sx*sl))^T)*2... i.e. WR + WR^T + C with the scale
folded into R.  Expansion is one 262144x16 @ 16x128 sgemm + transpose-add.

Accuracy: rel fro err ~7.3e-3 end-to-end (gate 2e-2), dominated by the x
e3m4 quantization (which is attenuated 8x by the rank-16 projection).
"""

import numpy as np

N_CORES = 8
BATCH = 2048
N = 128
K = 16                               # rank of A / W columns
PER_CORE = BATCH // N_CORES          # 256
GROUP = 16                           # SPD matrices per PSUM tile
N_GROUPS = PER_CORE // GROUP         # 16
EPS = 1e-4
SX = 4.0                             # x fp8 scale
SL = 16.0                            # L fp8 scale

_compiled = {}


def _host_consts(w_enc0, w_enc1, w_enc2, w_dec0, w_dec1, w_dec2):
    """Device consts (l8t fp8, a2t fp16) and host expansion mats (Rh, C)."""
    import ml_dtypes

    f8 = np.float64
    W0 = w_enc0[0, 0].astype(f8)     # (64,128)
    W1 = w_enc1[0, 0].astype(f8)     # (32,64)
    W2 = w_enc2[0, 0].astype(f8)     # (16,32)
    D0 = w_dec0[0, 0].astype(f8)     # (32,16)
    D1 = w_dec1[0, 0].astype(f8)     # (64,32)
    D2 = w_dec2[0, 0].astype(f8)     # (128,64)
    L = W2 @ W1 @ W0                 # (16,128)
    R = D2 @ D1 @ D0                 # (128,16)
    A = R @ L                        # (128,128) rank 16
    P1 = np.eye(32) - D0 @ D0.T
    P2 = np.eye(64) - D1 @ D1.T
    P3 = np.eye(128) - D2 @ D2.T
    C = EPS * (D2 @ (D1 @ P1 @ D1.T + P2) @ D2.T + P3)

    l8t = np.ascontiguousarray(SL * L.T).astype(np.float32).astype(
        ml_dtypes.float8_e3m4)                       # (128,16) fp8 device const
    Ltil = l8t.astype(np.float64).T / SL             # dequantized L~
    A2 = (2.0 * A - R @ Ltil).astype(np.float16)     # fp16, error-cancelling
    a2t = np.ascontiguousarray(A2.T)                 # (128,128) fp16
    # host expansion matrix: out = W @ Rh^T + (W @ Rh^T)^T + C
    # W carries scale SX*SL and we also need the 1/2 from sym():
    Rh = (R / (2.0 * SX * SL)).astype(np.float32)    # (128,16)
    return l8t, a2t, Rh, C.astype(np.float32)


def _build_bass(psum_bufs=4):
    import concourse.mybir as mybir
    from concourse import bacc
    from concourse.tile import TileContext

    W = GROUP * K                    # W8 tile width (128)
    WALL = PER_CORE * N              # full-core x width (32768)
    WOUT = PER_CORE * K              # full-core w width (4096)

    nc = bacc.Bacc(None, target_bir_lowering=False)
    f8e3 = mybir.dt.float8e3
    f16 = mybir.dt.float16
    f32 = mybir.dt.float32
    x = nc.dram_tensor("x", [N, WALL], f8e3, kind="ExternalInput")
    l8t = nc.dram_tensor("l8t", [N, K], f8e3, kind="ExternalInput")
    a2t = nc.dram_tensor("a2t", [N, N], f16, kind="ExternalInput")
    wout = nc.dram_tensor("w", [N, WOUT], f16, kind="ExternalOutput")

    # input slice schedule in matrices: fine first so compute starts early.
    # Issue is THROTTLED (~1-2 slices outstanding): the DMA rings round-robin
    # chunks of all outstanding transfers, so a deep backlog makes every
    # slice complete near the end and stalls compute on early matrices.
    in_sizes = [8, 8, 16] + [16] * 14
    # output slice schedule in groups (1 group = GROUP mats = GROUP*K cols)
    out_sizes_g = [8, 4, 2, 1, 1]
    assert sum(in_sizes) == PER_CORE and sum(out_sizes_g) == N_GROUPS

    with TileContext(nc) as tc:
        with (
            tc.tile_pool(name="consts", bufs=1) as cpool,
            tc.tile_pool(name="ysb", bufs=4) as ypool,
            tc.tile_pool(name="psy", bufs=psum_bufs, space="PSUM") as psy_pool,
            tc.tile_pool(name="pso", bufs=psum_bufs, space="PSUM") as pso_pool,
        ):
            l8t_sb = cpool.tile([N, K], f8e3)
            a2t_sb = cpool.tile([N, N], f16)
            nc.scalar.dma_start(out=l8t_sb, in_=l8t[:, :])
            nc.scalar.dma_start(out=a2t_sb, in_=a2t[:, :])
            xt = cpool.tile([N, WALL], f8e3)     # whole-core input (32KB/part)
            wt = cpool.tile([N, WOUT], f16)      # whole-core output (8KB/part)

            in_done = 0
            in_idx = 0
            LOOKAHEAD = 48                       # matrices issued ahead
            out_done = 0                         # groups drained
            next_out = out_sizes_g[0]
            oi = 0
            ysb_prev = None                      # software pipeline: mm2 lags
            for k in range(N_GROUPS + 1):
                if k < N_GROUPS:
                    while (in_done < PER_CORE
                           and in_done < (k + 1) * GROUP + LOOKAHEAD):
                        sz = in_sizes[in_idx]
                        a, b = in_done * N, (in_done + sz) * N
                        ieng = nc.sync if in_idx % 2 == 0 else nc.gpsimd
                        ieng.dma_start(out=xt[:, a:b], in_=x[:, a:b])
                        in_done += sz
                        in_idx += 1
                    psy = psy_pool.tile([N, W], f32, tag="psy")
                    for g in range(GROUP):
                        m = k * GROUP + g
                        nc.tensor.matmul(
                            psy[:, g * K:(g + 1) * K],
                            lhsT=xt[:, m * N:(m + 1) * N],
                            rhs=l8t_sb,
                            start=True, stop=True,
                        )
                # mm2 of the PREVIOUS group: its ysb is long ready, so the
                # PE never stalls on the PSUM evacuation of group k.
                if ysb_prev is not None:
                    pso = pso_pool.tile([N, W], f32, tag="pso")
                    nc.tensor.matmul(
                        pso, lhsT=a2t_sb, rhs=ysb_prev, start=True, stop=True,
                    )
                    nc.vector.tensor_copy(
                        wt[:, (k - 1) * W:k * W], pso)
                if k < N_GROUPS:
                    ysb = ypool.tile([N, W], f16, tag="ysb")
                    nc.scalar.copy(ysb, psy)
                    ysb_prev = ysb
                # drain completed output slices (group k-1 written above)
                while k >= out_done + next_out:
                    a, b = out_done * W, (out_done + next_out) * W
                    oeng = nc.sync if oi >= len(out_sizes_g) - 2 else nc.gpsimd
                    oeng.dma_start(out=wout[:, a:b], in_=wt[:, a:b])
                    out_done += next_out
                    oi += 1
                    next_out = out_sizes_g[oi] if oi < len(out_sizes_g) else N_GROUPS
    nc.compile()
    return nc


def _pack_x(x_full):
    """(BATCH,N,N) f32 -> per-core fp8 [N, PER_CORE*N] SBUF layout [p,(m c)]."""
    import ml_dtypes

    x8 = (x_full * np.float32(SX)).astype(ml_dtypes.float8_e3m4)
    x8 = x8.reshape(N_CORES, PER_CORE, N, N).transpose(0, 2, 1, 3)
    return np.ascontiguousarray(x8).reshape(N_CORES, N, PER_CORE * N)


def _get_nc():
    if "nc" not in _compiled:
        _compiled["nc"] = _build_bass()
    return _compiled["nc"]


def kernel(x, w_enc0, w_enc1, w_enc2, w_dec0, w_dec1, w_dec2, trace=False):
    from concourse.bass_utils import run_bass_kernel_spmd

    l8t, a2t, Rh, C = _host_consts(
        w_enc0, w_enc1, w_enc2, w_dec0, w_dec1, w_dec2)
    xs = np.asarray(x, dtype=np.float32).reshape(BATCH, N, N)
    xp = _pack_x(xs)

    nc = _get_nc()
    in_maps = [
        {"x": xp[i], "l8t": l8t, "a2t": a2t}
        for i in range(N_CORES)
    ]
    res = run_bass_kernel_spmd(nc, in_maps, core_ids=list(range(N_CORES)), trace=trace)
    # gather W: per core [N, PER_CORE*K] -> (BATCH, N, K)
    Wg = np.concatenate(
        [
            np.ascontiguousarray(
                r["w"].reshape(N, PER_CORE, K).transpose(1, 0, 2))
            for r in res.results
        ],
        axis=0,
    ).astype(np.float32)
    # host expansion: out = W Rh^T + (W Rh^T)^T + C   (scales folded into Rh)
    WR = (Wg.reshape(-1, K) @ Rh.T).reshape(BATCH, N, N)
    out = WR + WR.transpose(0, 2, 1)
    out += C
    if trace:
        _compiled["last_results"] = res
    return out.reshape(BATCH, 1, N, N)


# revision 22
# speedup vs baseline: 1.2111x; 1.0192x over previous
"""SPDnet autoencoder (nn_Autoencoder_layers_byhalf_SPDnet) on 8 trn2 NeuronCores.

Mathematical collapse (same as the A x A^T + C baseline, see below), plus a
rank-16 factorization that the device exploits:

  out[b] = A x[b] A^T + C,  A = R L  with  L (16x128) = W2 W1 W0 and
  R (128x16) = D2 D1 D0 both semi-orthogonal;  C is a host constant.

Because A has rank 16, the full product never needs to be formed on device:

  out = sym(A2 (x L~^T) R^T) + C        (up to quantization)

where L~ = dequant(fp8(sl*L))/sl and A2 = fp16(2A - R L~).  The choice
A2 = 2A - R L~ makes the first-order weight-quantization error cancel under
host symmetrization:  sym((A+d) x (A-d)^T) = A x A^T - d x d^T.

Device (per core, 256 SPD matrices, fp8 input):
  * Host packs x -> e3m4(4*x) in SBUF layout [p, (m c)]  (4.19 MB/core,
    half the fp16 baseline's input bytes).
  * Per group of 8 matrices: 8 tiny matmuls V'_b = x_b @ L8T (stationary
    x_b fp8 with auto FWL, moving L8T [128,16] -> stream 16 cycles), one
    whole-tile evac [128,128] f32->fp16, one batched matmul
    W8 = A2 @ [V'_0..V'_7] (stationary A2^T fp16 shared across the group,
    stream 128), one evac to the persistent output tile.
  * Output is only W (128x16 fp16 per matrix) = 1.05 MB/core, 8x less than
    the full symmetric output.
Host: out = sym(W (R/(2*sx*sl))^T)*2... i.e. WR + WR^T + C with the scale
folded into R.  Expansion is one 262144x16 @ 16x128 sgemm + transpose-add.

Accuracy: rel fro err ~7.3e-3 end-to-end (gate 2e-2), dominated by the x
e3m4 quantization (which is attenuated 8x by the rank-16 projection).
"""

import numpy as np

N_CORES = 8
BATCH = 2048
N = 128
K = 16                               # rank of A / W columns
PER_CORE = BATCH // N_CORES          # 256
GROUP = 16                           # SPD matrices per PSUM tile
N_GROUPS = PER_CORE // GROUP         # 16
EPS = 1e-4
SX = 4.0                             # x fp8 scale
SL = 16.0                            # L fp8 scale

_compiled = {}


def _host_consts(w_enc0, w_enc1, w_enc2, w_dec0, w_dec1, w_dec2):
    """Device consts (l8t fp8, a2t fp16) and host expansion mats (Rh, C)."""
    import ml_dtypes

    f8 = np.float64
    W0 = w_enc0[0, 0].astype(f8)     # (64,128)
    W1 = w_enc1[0, 0].astype(f8)     # (32,64)
    W2 = w_enc2[0, 0].astype(f8)     # (16,32)
    D0 = w_dec0[0, 0].astype(f8)     # (32,16)
    D1 = w_dec1[0, 0].astype(f8)     # (64,32)
    D2 = w_dec2[0, 0].astype(f8)     # (128,64)
    L = W2 @ W1 @ W0                 # (16,128)
    R = D2 @ D1 @ D0                 # (128,16)
    A = R @ L                        # (128,128) rank 16
    P1 = np.eye(32) - D0 @ D0.T
    P2 = np.eye(64) - D1 @ D1.T
    P3 = np.eye(128) - D2 @ D2.T
    C = EPS * (D2 @ (D1 @ P1 @ D1.T + P2) @ D2.T + P3)

    l8t = np.ascontiguousarray(SL * L.T).astype(np.float32).astype(
        ml_dtypes.float8_e3m4)                       # (128,16) fp8 device const
    Ltil = l8t.astype(np.float64).T / SL             # dequantized L~
    A2 = (2.0 * A - R @ Ltil).astype(np.float16)     # fp16, error-cancelling
    a2t = np.ascontiguousarray(A2.T)                 # (128,128) fp16
    # host expansion matrix: out = W @ Rh^T + (W @ Rh^T)^T + C
    # W carries scale SX*SL and we also need the 1/2 from sym():
    Rh = (R / (2.0 * SX * SL)).astype(np.float32)    # (128,16)
    return l8t, a2t, Rh, C.astype(np.float32)


def _build_bass(psum_bufs=3):
    import concourse.mybir as mybir
    from concourse import bacc
    from concourse.tile import TileContext

    W = GROUP * K                    # W8 tile width (128)
    WALL = PER_CORE * N              # full-core x width (32768)
    WOUT = PER_CORE * K              # full-core w width (4096)

    nc = bacc.Bacc(None, target_bir_lowering=False)
    f8e3 = mybir.dt.float8e3
    f16 = mybir.dt.float16
    f32 = mybir.dt.float32
    x = nc.dram_tensor("x", [N, WALL], f8e3, kind="ExternalInput")
    l8t = nc.dram_tensor("l8t", [N, K], f8e3, kind="ExternalInput")
    a2t = nc.dram_tensor("a2t", [N, N], f16, kind="ExternalInput")
    wout = nc.dram_tensor("w", [N, WOUT], f16, kind="ExternalOutput")

    # input slice schedule in matrices: fine first so compute starts early.
    # Issue is THROTTLED (~1-2 slices outstanding): the DMA rings round-robin
    # chunks of all outstanding transfers, so a deep backlog makes every
    # slice complete near the end and stalls compute on early matrices.
    in_sizes = [64, 48, 48, 48, 48]
    # output slice schedule in groups (1 group = GROUP mats = GROUP*K cols)
    out_sizes_g = [8, 4, 2, 1, 1]
    assert sum(in_sizes) == PER_CORE and sum(out_sizes_g) == N_GROUPS

    with TileContext(nc) as tc:
        with (
            tc.tile_pool(name="consts", bufs=1) as cpool,
            tc.tile_pool(name="ysb", bufs=4) as ypool,
            tc.tile_pool(name="psy", bufs=psum_bufs, space="PSUM") as psy_pool,
            tc.tile_pool(name="pso", bufs=psum_bufs, space="PSUM") as pso_pool,
            tc.tile_pool(name="warm", bufs=1, space="PSUM") as warm_pool,
        ):
            l8t_sb = cpool.tile([N, K], f8e3)
            a2t_sb = cpool.tile([N, N], f16)
            nc.scalar.dma_start(out=l8t_sb, in_=l8t[:, :])
            nc.scalar.dma_start(out=a2t_sb, in_=a2t[:, :])
            xt = cpool.tile([N, WALL], f8e3)     # whole-core input (32KB/part)
            wt = cpool.tile([N, WOUT], f16)      # whole-core output (8KB/part)

            # HAM warmup: the PE clock-gate stays at 1.2 GHz until it sees
            # ~3.4us of sustained activity; input-paced gaps otherwise keep
            # the whole kernel cold (2x slower PE).  Run ~3.5us of dummy
            # matmuls (junk SBUF data, scratch PSUM, no deps on real tiles)
            # while the first input slice is in flight.
            junk_sb = cpool.tile([N, 512], f8e3)
            nc.vector.memset(junk_sb, 0)
            warm_ps = warm_pool.tile([16, 512], f32)
            for _ in range(9):
                nc.tensor.matmul(
                    warm_ps, lhsT=l8t_sb, rhs=junk_sb[:, 0:512],
                    start=True, stop=True,
                )

            in_done = 0
            in_idx = 0
            LOOKAHEAD = 64                       # matrices issued ahead
            out_done = 0                         # groups drained
            next_out = out_sizes_g[0]
            oi = 0
            ysb_prev = None                      # software pipeline: mm2 lags
            for k in range(N_GROUPS + 1):
                if k < N_GROUPS:
                    while (in_done < PER_CORE
                           and in_done < (k + 1) * GROUP + LOOKAHEAD):
                        sz = in_sizes[in_idx]
                        a, b = in_done * N, (in_done + sz) * N
                        ieng = nc.sync if in_idx % 2 == 0 else nc.gpsimd
                        ieng.dma_start(out=xt[:, a:b], in_=x[:, a:b])
                        in_done += sz
                        in_idx += 1
                    psy = psy_pool.tile([N, W], f32, tag="psy")
                    for g in range(GROUP):
                        m = k * GROUP + g
                        nc.tensor.matmul(
                            psy[:, g * K:(g + 1) * K],
                            lhsT=xt[:, m * N:(m + 1) * N],
                            rhs=l8t_sb,
                            start=True, stop=True,
                        )
                # mm2 of the PREVIOUS group: its ysb is long ready, so the
                # PE never stalls on the PSUM evacuation of group k.
                if ysb_prev is not None:
                    pso = pso_pool.tile([N, W], f32, tag="pso")
                    nc.tensor.matmul(
                        pso, lhsT=a2t_sb, rhs=ysb_prev, start=True, stop=True,
                    )
                    nc.vector.tensor_copy(
                        wt[:, (k - 1) * W:k * W], pso)
                if k < N_GROUPS:
                    ysb = ypool.tile([N, W], f16, tag="ysb")
                    nc.scalar.copy(ysb, psy)
                    ysb_prev = ysb
                # drain completed output slices (group k-1 written above)
                while k >= out_done + next_out:
                    a, b = out_done * W, (out_done + next_out) * W
                    oeng = nc.sync if oi >= len(out_sizes_g) - 2 else nc.gpsimd
                    oeng.dma_start(out=wout[:, a:b], in_=wt[:, a:b])
                    out_done += next_out
                    oi += 1
                    next_out = out_sizes_g[oi] if oi < len(out_sizes_g) else N_GROUPS
    nc.compile()
    return nc


def _pack_x(x_full):
    """(BATCH,N,N) f32 -> per-core fp8 [N, PER_CORE*N] SBUF layout [p,(m c)]."""
    import ml_dtypes

    x8 = (x_full * np.float32(SX)).astype(ml_dtypes.float8_e3m4)
    x8 = x8.reshape(N_CORES, PER_CORE, N, N).transpose(0, 2, 1, 3)
    return np.ascontiguousarray(x8).reshape(N_CORES, N, PER_CORE * N)


def _get_nc():
    if "nc" not in _compiled:
        _compiled["nc"] = _build_bass()
    return _compiled["nc"]


def kernel(x, w_enc0, w_enc1, w_enc2, w_dec0, w_dec1, w_dec2, trace=False):
    from concourse.bass_utils import run_bass_kernel_spmd

    l8t, a2t, Rh, C = _host_consts(
        w_enc0, w_enc1, w_enc2, w_dec0, w_dec1, w_dec2)
    xs = np.asarray(x, dtype=np.float32).reshape(BATCH, N, N)
    xp = _pack_x(xs)

    nc = _get_nc()
    in_maps = [
        {"x": xp[i], "l8t": l8t, "a2t": a2t}
        for i in range(N_CORES)
    ]
    res = run_bass_kernel_spmd(nc, in_maps, core_ids=list(range(N_CORES)), trace=trace)
    # gather W: per core [N, PER_CORE*K] -> (BATCH, N, K)
    Wg = np.concatenate(
        [
            np.ascontiguousarray(
                r["w"].reshape(N, PER_CORE, K).transpose(1, 0, 2))
            for r in res.results
        ],
        axis=0,
    ).astype(np.float32)
    # host expansion: out = W Rh^T + (W Rh^T)^T + C   (scales folded into Rh)
    WR = (Wg.reshape(-1, K) @ Rh.T).reshape(BATCH, N, N)
    out = WR + WR.transpose(0, 2, 1)
    out += C
    if trace:
        _compiled["last_results"] = res
    return out.reshape(BATCH, 1, N, N)
